# revision 3
# baseline (speedup 1.0000x reference)
"""nn_Attention_50749333569807 Bass/Tile kernel for 8 Trainium2 NeuronCores (v2).

Model: x -> 1x1 conv (qkv) -> depthwise 3x3 -> channel attention
(L2-normalized q,k over spatial, softmax over key channels) -> 1x1 proj.

Sharding: 8 cores = 4 batches x 2 spatial halves (64 image rows each).
Halo rows come from the host (zeros at image borders, neighbor rows at
the half boundary).

v2 changes vs v1:
  - depthwise taps are multi-row strided matmuls (3D APs): ~9-15 PE
    matmuls per (ct, chunk) instead of 36; 3 taps (dy=+1 row) ride on
    DVE/Pool via fused scalar_tensor_tensor chains.
  - v never bounces through DRAM: the qkv GEMM evacuates v into a
    persistent SBUF buffer [128, 9216] per v-ctile; the depthwise for v
    runs *during the AllReduce bubble*, writing in place one 4-row slot
    to the left (reads stay strictly ahead of writes).
  - attention is folded into the projection: M = Wproj @ A (dense
    384x384, head straddles included) is built once on PE after
    softmax; the output loop is just 9 accumulating matmuls per chunk
    of M^T against the shifted v buffer. No attn@v stage, no pair
    layout, no ao evacuations.
  - gram/sumsq AllReduce is split in halves: chunks 0-7 ship after
    chunk 7 and overlap the rest of the main loop; only the second
    half's latency is exposed, and the v-depthwise fills that bubble.
  - q,k transposes ride the Activation HWDGE queue, bulk DMA the SP
    queue.
"""

import contextlib
import numpy as np
import ml_dtypes

B, DIM, H, W = 4, 384, 128, 128
HEADS = 8
CH = DIM // HEADS  # 48
QKV = 3 * DIM  # 1152
ROWS_LOC = 66
NLOC = ROWS_LOC * W  # 8448
NOWN = 64 * W  # 8192
N_CORES = 8
GCHUNKS = 17  # 16 x 4 rows + 1 x 2 rows
OCHUNKS = 16
EPS = 1e-12

# PE taps (first must be (0,0): full-tile PSUM init). Chain taps are the
# dy=-1 row: single-run on the j-aligned chunking, so DVE does only 3 ops
# per (ct, chunk); the first chain tap has dx == 0 (full chain init).
# Pool taps (broadcast-weight tensor_tensor) measured ~4x slower on HW
# than the sim models — keep Pool free of tap work.
PE_TAPS = [(0, 0), (0, -1), (0, 1), (1, -1), (1, 1), (1, 0)]
CHAIN_TAPS = [(-1, 0), (-1, -1), (-1, 1)]
POOL_TAPS = []
NPE = len(PE_TAPS)

_CACHE = {}
REPLICATE = 1


def _gemm_chunk_cols(g):
    return 512 if g < 16 else 256


def split_multiwaits(nc, max_waits=1):
    """This container's walrus accepts a single sync-wait per instruction;
    split extras into single-wait NOPs placed before the instruction."""
    import concourse.mybir as mybir

    for f in nc.m.functions:
        for bb in f.blocks:
            insts = bb.instructions
            out = []
            for ins in insts:
                si = ins.sync_info
                if si is not None and si.on_wait and len(si.on_wait) > max_waits:
                    waits = list(si.on_wait)
                    for w in waits[:-max_waits]:
                        nop = mybir.InstNoOp(
                            name=nc.get_next_instruction_name(),
                            engine=ins.engine,
                            ins=[], outs=[],
                            sync_info=mybir.SyncInfo(on_wait=[w], on_update=[]),
                        )
                        out.append(nop)
                    ins.sync_info = mybir.SyncInfo(
                        on_wait=waits[-max_waits:], on_update=list(si.on_update)
                    )
                out.append(ins)
            insts[:] = out


def _build_nc():
    import concourse.bass as bass
    import concourse.mybir as mb
    import concourse.tile as tile
    from concourse.masks import make_identity

    f32 = mb.dt.float32
    bf16 = mb.dt.bfloat16
    Act = mb.ActivationFunctionType
    Alu = mb.AluOpType

    nc = bass.Bass("TRN2", num_devices=N_CORES)

    fp8 = mb.dt.float8e4

    xl_d = nc.declare_dram_parameter("xl", [DIM, NLOC], bf16, isOutput=False)
    xl8_d = nc.declare_dram_parameter("xl8", [DIM, NLOC], fp8, isOutput=False)
    wqdr_d = nc.declare_dram_parameter("wqdr", [128, 6 * 256], fp8, isOutput=False)
    wq2_d = nc.declare_dram_parameter("wq2", [128, 6 * 128], fp8, isOutput=False)
    wqv_d = nc.declare_dram_parameter("wqv", [DIM, DIM], bf16, isOutput=False)
    wdiag_d = nc.declare_dram_parameter(
        "wdiag", [128, 81 * 128], bf16, isOutput=False)
    wdwps_d = nc.declare_dram_parameter("wdwps", [128, 81], f32, isOutput=False)
    wprojT_d = nc.declare_dram_parameter("wprojT", [DIM, DIM], bf16, isOutput=False)
    tmprow_d = nc.declare_dram_parameter("tmprow", [1, DIM], f32, isOutput=False)
    yout_d = nc.declare_dram_parameter("yout", [DIM, NOWN], bf16, isOutput=True)

    def rloc(r):  # local row -> (gemm chunk, row-within-chunk)
        return r // 4, r % 4

    with tile.TileContext(nc) as tc, contextlib.ExitStack() as ctx:
        singles = ctx.enter_context(tc.tile_pool(name="singles", bufs=1))
        xpool = ctx.enter_context(tc.tile_pool(name="xpool", bufs=4))
        prep = ctx.enter_context(tc.tile_pool(name="prep", bufs=3))
        dwp = ctx.enter_context(tc.tile_pool(name="dwp", bufs=2))
        qkTp = ctx.enter_context(tc.tile_pool(name="qkTp", bufs=2))
        chainp = ctx.enter_context(tc.tile_pool(name="chainp", bufs=2))
        smalls = ctx.enter_context(tc.tile_pool(name="smalls", bufs=1))
        outp = ctx.enter_context(tc.tile_pool(name="outp", bufs=3))
        dramp = ctx.enter_context(tc.tile_pool(name="dramp", bufs=1, space="DRAM"))

        cc1_in = dramp.tile([128, 488], f32, name="cc1_in")
        cc1_out = dramp.tile([128, 488], f32, name="cc1_out")

        # ---- persistent loads ----
        wqdr_sb = singles.tile([128, 6 * 256], fp8, name="wqdr")
        nc.sync.dma_start(out=wqdr_sb, in_=wqdr_d.ap())
        wq2_sb = singles.tile([128, 6 * 128], fp8, name="wq2")
        nc.sync.dma_start(out=wq2_sb, in_=wq2_d.ap())
        wqv_sb = []
        for kc in range(3):
            t = singles.tile([128, DIM], bf16, name=f"wqv{kc}")
            nc.sync.dma_start(out=t, in_=wqv_d.ap()[kc * 128:(kc + 1) * 128, :])
            wqv_sb.append(t)
        wdiag_sb = singles.tile([128, 81 * 128], bf16, name="wdiag")
        nc.scalar.dma_start(out=wdiag_sb, in_=wdiag_d.ap())
        wdwps_sb = singles.tile([128, 81], f32, name="wdwps")
        nc.scalar.dma_start(out=wdwps_sb, in_=wdwps_d.ap())
        wproj_sb = []
        for ct in range(3):
            t = singles.tile([128, DIM], bf16, name=f"wprojT{ct}")
            nc.scalar.dma_start(out=t, in_=wprojT_d.ap()[ct * 128:(ct + 1) * 128, :])
            wproj_sb.append(t)
        tmprow_sb = singles.tile([1, DIM], f32, name="tmprow")
        nc.scalar.dma_start(out=tmprow_sb, in_=tmprow_d.ap())
        ident_f32 = singles.tile([128, 128], f32, name="ident_f32")
        make_identity(nc, ident_f32)

        ssq_slots = singles.tile([128, 6, OCHUNKS], f32, name="ssq_slots")
        sq_scratch = singles.tile([128, 512], bf16, name="sq_scratch")

        # persistent v buffer: row r of v0 at col 512 + 128*r; after the
        # in-place depthwise, dw(v) row r lands at col 128*r.
        v0big = [singles.tile([128, 9216], bf16, name=f"v0big{vc}")
                 for vc in range(3)]

        def one_pass():
            x_tiles = {}
            pre_tiles = {}
            qkT_tiles = {}

            def load_x(g):
                ncols = _gemm_chunk_cols(g)
                for kc in range(3):
                    t = xpool.tile([128, 512], bf16, tag=f"x{kc}", name=f"xt{kc}_{g}")
                    nc.sync.dma_start(
                        out=t[:, :ncols],
                        in_=xl_d.ap()[kc * 128:(kc + 1) * 128, g * 512:g * 512 + ncols],
                    )
                    x_tiles[(kc, g)] = t
                # fp8 copies for the q,k GEMM: kc 0,1 side by side (Ko step
                # 512 for DoubleRow), kc2 separate
                t8 = xpool.tile([128, 1024], fp8, tag="x8", name=f"x8_{g}")
                for e in range(2):
                    nc.sync.dma_start(
                        out=t8[:, e * 512:e * 512 + ncols],
                        in_=xl8_d.ap()[e * 128:(e + 1) * 128, g * 512:g * 512 + ncols],
                    )
                x_tiles[("dr", g)] = t8
                t82 = xpool.tile([128, 512], fp8, tag="x82", name=f"x82_{g}")
                nc.sync.dma_start(
                    out=t82[:, :ncols],
                    in_=xl8_d.ap()[256:384, g * 512:g * 512 + ncols],
                )
                x_tiles[("k2", g)] = t82

            with contextlib.ExitStack() as psctx:
                ps_gemm = psctx.enter_context(
                    tc.tile_pool(name="ps_gemm", bufs=2, space="PSUM"))
                ps_dw = psctx.enter_context(
                    tc.tile_pool(name="ps_dw", bufs=2, space="PSUM"))
                ps_gram = psctx.enter_context(
                    tc.tile_pool(name="ps_gram", bufs=2, space="PSUM"))

                gram_ps = ps_gram.tile([128, 480], f32, tag="gram", name="gram_ps")
                # straddle slots: s0=(0,1)[32x16]@384, s1=(1,0)[16x32]@400,
                #                 s2=(1,2)[16x32]@432, s3=(2,1)[32x16]@464

                def gemm_chunk(g):
                    ncols = _gemm_chunk_cols(g)
                    for ot in range(9):
                        ps = ps_gemm.tile([128, 512], f32, tag="gemm", name=f"gps{ot}_{g}")
                        if ot < 6:
                            nc.tensor.matmul(
                                ps[:, :ncols],
                                wqdr_sb[:, ot * 256:(ot + 1) * 256].rearrange(
                                    "p (k m) -> p k m", k=2),
                                x_tiles[("dr", g)].rearrange(
                                    "p (k n) -> p k n", k=2)[:, :, :ncols],
                                start=True, stop=False,
                                perf_mode=mb.MatmulPerfMode.DoubleRow,
                            )
                            nc.tensor.matmul(
                                ps[:, :ncols],
                                wq2_sb[:, ot * 128:(ot + 1) * 128],
                                x_tiles[("k2", g)][:, :ncols],
                                start=False, stop=True,
                            )
                        else:
                            for kc in range(3):
                                nc.tensor.matmul(
                                    ps[:, :ncols],
                                    wqv_sb[kc][:, (ot - 6) * 128:(ot - 5) * 128],
                                    x_tiles[(kc, g)][:, :ncols],
                                    start=(kc == 0), stop=(kc == 2),
                                )
                        if ot < 6:
                            pre = prep.tile([128, 512], bf16, tag=f"pre{ot}",
                                            name=f"pre{ot}_{g}")
                            nc.scalar.copy(pre[:, :ncols], ps[:, :ncols])
                            pre_tiles[(ot, g)] = pre
                        else:
                            nc.scalar.copy(
                                v0big[ot - 6][:, 512 + g * 512:512 + g * 512 + ncols],
                                ps[:, :ncols],
                            )
                    for kc in range(3):
                        del x_tiles[(kc, g)]
                    del x_tiles[("dr", g)], x_tiles[("k2", g)]

                def tap_runs(r0, nrows, dy):
                    """Contiguous source runs for output rows r0..r0+nrows-1
                    shifted dy; returns [(rr, nrun, g, wsr)]."""
                    runs, rr = [], 0
                    while rr < nrows:
                        g, wsr = rloc(r0 + rr + dy)
                        nrun = 1
                        while (rr + nrun < nrows
                               and rloc(r0 + rr + nrun + dy)[0] == g):
                            nrun += 1
                        runs.append((rr, nrun, g, wsr))
                        rr += nrun
                    return runs

                def dwqk_chunk(ct, j):
                    """depthwise for q/k ctile ct, output rows 4j+1..4j+4."""
                    r0 = 1 + 4 * j
                    psd = ps_dw.tile([128, 512], f32, tag="dw", name=f"dps{ct}_{j}")
                    psd3 = psd.rearrange("p (r w) -> p r w", w=128)
                    mms = []
                    for ti, (dy, dx) in enumerate(PE_TAPS):
                        blk = (ct * 9 + 3 * (dy + 1) + (dx + 1)) * 128
                        iw0, iw1 = max(0, dx), 128 + min(0, dx)
                        ow0, ow1 = max(0, -dx), 128 + min(0, -dx)
                        for (rr, nrun, g, wsr) in tap_runs(r0, 4, dy):
                            src = pre_tiles[(ct, g)].rearrange(
                                "p (r w) -> p r w", w=128)[:, wsr:wsr + nrun, iw0:iw1]
                            mms.append((
                                psd3[:, rr:rr + nrun, ow0:ow1],
                                wdiag_sb[:, blk:blk + 128], src, ti == 0,
                            ))
                    for k, (o, lt, rt, st) in enumerate(mms):
                        nc.tensor.matmul(o, lt, rt, start=st,
                                         stop=(k == len(mms) - 1))
                    # chain taps + combine on DVE (Pool can't touch PSUM or
                    # run TensorScalarPtr); evacs/Squares on ACT.
                    eng = nc.vector
                    ceng = nc.vector
                    chain = chainp.tile([128, 4, 128], bf16, tag=f"ch{ct}",
                                        name=f"ch{ct}_{j}")
                    for kd, (dy, dx) in enumerate(CHAIN_TAPS):
                        tap = 3 * (dy + 1) + (dx + 1)
                        sc = wdwps_sb[:, ct * 9 + tap:ct * 9 + tap + 1]
                        iw0, iw1 = max(0, dx), 128 + min(0, dx)
                        ow0, ow1 = max(0, -dx), 128 + min(0, -dx)
                        if kd == 0:
                            assert dx == 0
                        for (rr, nrun, g, wsr) in tap_runs(r0, 4, dy):
                            src = pre_tiles[(ct, g)].rearrange(
                                "p (r w) -> p r w", w=128)[:, wsr:wsr + nrun, iw0:iw1]
                            dst = chain[:, rr:rr + nrun, ow0:ow1]
                            if kd == 0:
                                eng.tensor_scalar(dst, src, sc, None, op0=Alu.mult)
                            else:
                                eng.scalar_tensor_tensor(
                                    dst, src, sc, dst, op0=Alu.mult, op1=Alu.add)
                    dw = dwp.tile([128, 512], bf16, tag=f"dw{ct}", name=f"dw{ct}_{j}")
                    ceng.tensor_add(dw, psd, chain.rearrange("p r w -> p (r w)"))
                    # Pool taps: product into scratch, then in-place add onto
                    # dw (Pool can't see PSUM, so these ride after the combine)
                    dw3 = dw.rearrange("p (r w) -> p r w", w=128)
                    pt = chainp.tile([128, 4, 128], bf16, tag=f"pt{ct}",
                                     name=f"pt{ct}_{j}")
                    for dy, dx in POOL_TAPS:
                        tap = 3 * (dy + 1) + (dx + 1)
                        sc = wdwps_sb[:, ct * 9 + tap:ct * 9 + tap + 1]
                        iw0, iw1 = max(0, dx), 128 + min(0, dx)
                        ow0, ow1 = max(0, -dx), 128 + min(0, -dx)
                        for (rr, nrun, g, wsr) in tap_runs(r0, 4, dy):
                            src = pre_tiles[(ct, g)].rearrange(
                                "p (r w) -> p r w", w=128)[:, wsr:wsr + nrun, iw0:iw1]
                            nc.gpsimd.tensor_tensor(
                                pt[:, rr:rr + nrun, ow0:ow1], src,
                                sc.broadcast_to([128, nrun, iw1 - iw0]), op=Alu.mult)
                            nc.gpsimd.tensor_tensor(
                                dw3[:, rr:rr + nrun, ow0:ow1],
                                dw3[:, rr:rr + nrun, ow0:ow1],
                                pt[:, rr:rr + nrun, ow0:ow1], op=Alu.add)
                    return dw

                def gram_chunk(j):
                    qT = [qkT_tiles[(ct, j)] for ct in range(3)]
                    kT = [qkT_tiles[(ct, j)] for ct in range(3, 6)]
                    gps = gram_ps
                    for nb in range(4):
                        st = (j == 0 and nb == 0)
                        fin = (j == OCHUNKS - 1 and nb == 3)
                        mms = [
                            (gps[:, 0:128], qT[0][:, nb, :], kT[0][:, nb, :]),
                            (gps[:, 128:256], qT[1][:, nb, :], kT[1][:, nb, :]),
                            (gps[:, 256:384], qT[2][:, nb, :], kT[2][:, nb, :]),
                            (gps[0:32, 384:400], qT[0][:, nb, 96:128], kT[1][:, nb, 0:16]),
                            (gps[0:16, 400:432], qT[1][:, nb, 0:16], kT[0][:, nb, 96:128]),
                            (gps[0:16, 432:464], qT[1][:, nb, 112:128], kT[2][:, nb, 0:32]),
                            (gps[0:32, 464:480], qT[2][:, nb, 0:32], kT[1][:, nb, 112:128]),
                        ]
                        for mi, (o, lt, rt) in enumerate(mms):
                            nc.tensor.matmul(
                                o, lt, rt, start=st, stop=(fin and mi == len(mms) - 1)
                            )
                            st = False

                load_x(0)
                load_x(1)
                gemm_chunk(0)
                for j in range(OCHUNKS):
                    if j + 1 < GCHUNKS:
                        if j + 2 < GCHUNKS:
                            load_x(j + 2)
                        gemm_chunk(j + 1)
                    for ct in range(6):
                        dw = dwqk_chunk(ct, j)
                        nc.scalar.activation(
                            sq_scratch, dw, func=Act.Square,
                            accum_out=ssq_slots[:, ct, j:j + 1],
                        )
                        qkt = qkTp.tile([128, 4, 128], bf16, tag=f"qkT{ct}",
                                        name=f"qkT{ct}_{j}")
                        nc.sync.dma_start_transpose(out=qkt[:], in_=dw)
                        qkT_tiles[(ct, j)] = qkt
                    gram_chunk(j)
                    for ct in range(6):
                        del qkT_tiles[(ct, j)]

                # ---- sumsq finish + ship partials ----
                ssq_sum = smalls.tile([128, 6], f32, name="ssq_sum")
                nc.vector.tensor_reduce(
                    ssq_sum, ssq_slots, axis=mb.AxisListType.X, op=Alu.add
                )
                gB_sb = smalls.tile([128, 488], f32, name="gB_sb")
                nc.vector.tensor_copy(gB_sb[:, 0:480], gram_ps)
                nc.vector.tensor_copy(gB_sb[:, 480:486], ssq_sum)
                nc.vector.memset(gB_sb[:, 486:488], 0.0)
                nc.sync.dma_start(out=cc1_in, in_=gB_sb)

            # single AllReduce; its latency is hidden by the v depthwise,
            # which needs no transposes (collectives serialize with xbar
            # transposes) and no Pool work (the collective blocks Pool).
            nc.gpsimd.collective_compute(
                "AllReduce", Alu.add,
                replica_groups=[[0, 1], [2, 3], [4, 5], [6, 7]],
                ins=[cc1_in], outs=[cc1_out],
            )

            with contextlib.ExitStack() as psctx2:
                ps_v = psctx2.enter_context(
                    tc.tile_pool(name="ps_v", bufs=2, space="PSUM"))
                ps_sm = psctx2.enter_context(
                    tc.tile_pool(name="ps_sm", bufs=1, space="PSUM"))
                ps_out = psctx2.enter_context(
                    tc.tile_pool(name="ps_out", bufs=3, space="PSUM"))

                # ---- depthwise for v, in place during the AllReduce ----
                # v0 row r at col 512+128r; dw(v) row r written to col 128r.
                # 7 taps on PE, 2 on DVE accumulating straight into PSUM (the
                # PE sum is already there, so no init ordering is needed),
                # evacuated by ACT. Pool only carries the collective here.
                PE_TAPS_V = [(0, 0), (0, -1), (0, 1), (-1, -1), (-1, 0),
                             (-1, 1)]
                CHAIN_TAPS_V = [(1, -1), (1, 1), (1, 0)]

                def vdw_chunk(vc, g):
                    ct = 6 + vc
                    rows = list(range(4 * g, min(4 * g + 4, ROWS_LOC)))
                    nr = len(rows)
                    ncols = nr * 128
                    psv = ps_v.tile([128, 512], f32, tag="v", name=f"vps{vc}_{g}")
                    psv3 = psv.rearrange("p (r w) -> p r w", w=128)
                    src3 = v0big[vc].rearrange("p (r w) -> p r w", w=128)
                    mms = []
                    for ti, (dy, dx) in enumerate(PE_TAPS_V):
                        blk = (ct * 9 + 3 * (dy + 1) + (dx + 1)) * 128
                        iw0, iw1 = max(0, dx), 128 + min(0, dx)
                        ow0, ow1 = max(0, -dx), 128 + min(0, -dx)
                        sr0 = max(rows[0] + dy, 0)
                        sr1 = min(rows[-1] + dy, ROWS_LOC - 1)
                        if sr1 < sr0:
                            continue
                        orow = sr0 - dy - rows[0]
                        n = sr1 - sr0 + 1
                        mms.append((
                            psv3[:, orow:orow + n, ow0:ow1],
                            wdiag_sb[:, blk:blk + 128],
                            src3[:, 4 + sr0:4 + sr0 + n, iw0:iw1],
                            ti == 0,
                        ))
                    for k, (o, lt, rt, st) in enumerate(mms):
                        nc.tensor.matmul(o, lt, rt, start=st,
                                         stop=(k == len(mms) - 1))
                    for (dy, dx) in CHAIN_TAPS_V:
                        tap = 3 * (dy + 1) + (dx + 1)
                        sc = wdwps_sb[:, ct * 9 + tap:ct * 9 + tap + 1]
                        iw0, iw1 = max(0, dx), 128 + min(0, dx)
                        ow0, ow1 = max(0, -dx), 128 + min(0, -dx)
                        sr0 = max(rows[0] + dy, 0)
                        sr1 = min(rows[-1] + dy, ROWS_LOC - 1)
                        if sr1 < sr0:
                            continue
                        orow = sr0 - dy - rows[0]
                        n = sr1 - sr0 + 1
                        src = src3[:, 4 + sr0:4 + sr0 + n, iw0:iw1]
                        dst = psv3[:, orow:orow + n, ow0:ow1]
                        nc.vector.scalar_tensor_tensor(
                            dst, src, sc, dst, op0=Alu.mult, op1=Alu.add)
                    nc.scalar.copy(
                        v0big[vc][:, 4 * g * 128:4 * g * 128 + ncols],
                        psv[:, :ncols],
                    )

                for g in range(GCHUNKS):
                    for vc in range(3):
                        vdw_chunk(vc, g)

                # ---- receive ----
                ar_sb = smalls.tile([128, 488], f32, name="ar_sb")
                nc.sync.dma_start(out=ar_sb, in_=cc1_out)

                # ---- norms ----
                ssq_all = ar_sb[:, 480:486]
                n0 = smalls.tile([128, 6], f32, name="n0")
                nc.scalar.activation(n0, ssq_all, func=Act.Sqrt)
                rn0 = smalls.tile([128, 6], f32, name="rn0")
                nc.vector.reciprocal(rn0, n0)
                n1 = smalls.tile([128, 6], f32, name="n1")
                nc.vector.tensor_mul(n1, ssq_all, rn0)
                nc.vector.tensor_add(n1, n1, n0)
                nc.vector.tensor_scalar(n1, n1, 0.5, EPS, op0=Alu.mult, op1=Alu.max)
                rinv = smalls.tile([128, 6], f32, name="rinv")
                nc.vector.reciprocal(rinv, n1)

                rr_ps = ps_sm.tile([1, 768], f32, tag="sm", name="rr_ps")
                for ctt in range(6):
                    nc.tensor.matmul(
                        rr_ps[0:1, ctt * 128:(ctt + 1) * 128],
                        rinv[:, ctt:ctt + 1], ident_f32,
                        start=(ctt in (0, 4)), stop=(ctt in (3, 5)),
                    )
                rrow = smalls.tile([1, 768], f32, name="rrow")
                nc.vector.tensor_copy(rrow, rr_ps)
                nc.vector.tensor_mul(rrow[0:1, 0:DIM], rrow[0:1, 0:DIM], tmprow_sb)

                op_ps = ps_sm.tile([48, HEADS * CH], f32, tag="sm", name="op_ps")
                for h in range(HEADS):
                    nc.tensor.matmul(
                        op_ps[:, h * CH:(h + 1) * CH],
                        rrow[0:1, h * CH:(h + 1) * CH],
                        rrow[0:1, DIM + h * CH:DIM + (h + 1) * CH],
                        start=(h == 0), stop=(h == HEADS - 1),
                    )
                op_sb = smalls.tile([48, HEADS * CH], f32, name="op_sb")
                nc.vector.tensor_copy(op_sb, op_ps)

                # ---- per-head gram gather ----
                sm_in = smalls.tile([48, HEADS * CH], f32, name="sm_in")
                for h in range(HEADS):
                    i, o = (CH * h) // 128, (CH * h) % 128
                    if o + CH <= 128:
                        nc.sync.dma_start(
                            out=sm_in[:, h * CH:(h + 1) * CH],
                            in_=ar_sb[o:o + CH, i * 128 + o:i * 128 + o + CH],
                        )
                    elif h == 2:
                        nc.sync.dma_start(out=sm_in[0:32, h * CH:h * CH + 32],
                                          in_=ar_sb[96:128, 96:128])
                        nc.sync.dma_start(out=sm_in[0:32, h * CH + 32:h * CH + 48],
                                          in_=ar_sb[0:32, 384:400])
                        nc.sync.dma_start(out=sm_in[32:48, h * CH:h * CH + 32],
                                          in_=ar_sb[0:16, 400:432])
                        nc.sync.dma_start(out=sm_in[32:48, h * CH + 32:h * CH + 48],
                                          in_=ar_sb[0:16, 128:144])
                    else:  # h == 5
                        nc.sync.dma_start(out=sm_in[0:16, h * CH:h * CH + 16],
                                          in_=ar_sb[112:128, 240:256])
                        nc.sync.dma_start(out=sm_in[0:16, h * CH + 16:h * CH + 48],
                                          in_=ar_sb[0:16, 432:464])
                        nc.sync.dma_start(out=sm_in[16:48, h * CH:h * CH + 16],
                                          in_=ar_sb[0:32, 464:480])
                        nc.sync.dma_start(out=sm_in[16:48, h * CH + 16:h * CH + 48],
                                          in_=ar_sb[0:32, 256:288])

                # ---- softmax ----
                lg = smalls.tile([48, HEADS * CH], f32, name="lg")
                nc.vector.tensor_mul(lg, sm_in, op_sb)
                mx = smalls.tile([48, HEADS], f32, name="mx")
                nc.vector.tensor_reduce(
                    mx, lg.rearrange("p (h c) -> p h c", h=HEADS),
                    axis=mb.AxisListType.X, op=Alu.max,
                )
                for h in range(HEADS):
                    nc.vector.tensor_scalar(
                        lg[:, h * CH:(h + 1) * CH], lg[:, h * CH:(h + 1) * CH],
                        mx[:, h:h + 1], None, op0=Alu.subtract,
                    )
                nc.scalar.activation(lg, lg, func=Act.Exp)
                sm_sum = smalls.tile([48, HEADS], f32, name="sm_sum")
                nc.vector.tensor_reduce(
                    sm_sum, lg.rearrange("p (h c) -> p h c", h=HEADS),
                    axis=mb.AxisListType.X, op=Alu.add,
                )
                rsum = smalls.tile([48, HEADS], f32, name="rsum")
                nc.vector.reciprocal(rsum, sm_sum)
                attn = smalls.tile([48, HEADS * CH], bf16, name="attn")
                for h in range(HEADS):
                    nc.vector.tensor_scalar(
                        attn[:, h * CH:(h + 1) * CH], lg[:, h * CH:(h + 1) * CH],
                        rsum[:, h:h + 1], None, op0=Alu.mult,
                    )

                # ---- A assembly (ctile layout) + M^T = (Wproj @ A)^T ----
                A_sb = []
                for ct in range(3):
                    t = smalls.tile([128, DIM], bf16, name=f"A{ct}")
                    nc.vector.memset(t, 0.0)
                    A_sb.append(t)
                for h in range(HEADS):
                    c0 = h * CH
                    ct, o = c0 // 128, c0 % 128
                    cols = (c0, c0 + CH)
                    if o + CH <= 128:
                        nc.sync.dma_start(
                            out=A_sb[ct][o:o + CH, cols[0]:cols[1]],
                            in_=attn[0:CH, cols[0]:cols[1]])
                    else:
                        n1r = 128 - o
                        nc.sync.dma_start(
                            out=A_sb[ct][o:128, cols[0]:cols[1]],
                            in_=attn[0:n1r, cols[0]:cols[1]])
                        nc.sync.dma_start(
                            out=A_sb[ct + 1][0:CH - n1r, cols[0]:cols[1]],
                            in_=attn[n1r:CH, cols[0]:cols[1]])

                mt_sb = []
                for ct2 in range(3):
                    pm = ps_sm.tile([128, DIM], f32, tag="sm", name=f"mt_ps{ct2}")
                    for ct1 in range(3):
                        nc.tensor.matmul(
                            pm, A_sb[ct1][:, ct2 * 128:(ct2 + 1) * 128],
                            wproj_sb[ct1],
                            start=(ct1 == 0), stop=(ct1 == 2),
                        )
                    t = smalls.tile([128, DIM], bf16, name=f"mt{ct2}")
                    nc.scalar.copy(t, pm)
                    mt_sb.append(t)

                # ---- output: yout chunk jj = M @ dw(v) rows 4jj+1..4jj+4 ----
                for jj in range(OCHUNKS):
                    c0 = jj * 512 + 128
                    for mt in range(3):
                        po = ps_out.tile([128, 512], f32, tag="out", name=f"ops{mt}_{jj}")
                        for ct in range(3):
                            nc.tensor.matmul(
                                po, mt_sb[ct][:, mt * 128:(mt + 1) * 128],
                                v0big[ct][:, c0:c0 + 512],
                                start=(ct == 0), stop=(ct == 2),
                            )
                        ot = outp.tile([128, 512], bf16, tag=f"out{mt}", name=f"ot{mt}_{jj}")
                        # DVE is idle in the output phase; share evacuations
                        if mt == 0:
                            nc.vector.tensor_copy(ot, po)
                        else:
                            nc.scalar.copy(ot, po)
                        nc.sync.dma_start(
                            out=yout_d.ap()[mt * 128:(mt + 1) * 128,
                                            jj * 512:(jj + 1) * 512],
                            in_=ot,
                        )

        for _rep in range(REPLICATE):
            one_pass()
    return nc


def _prepare_shared_weights(w_qkv, w_dw, w_proj, temperature):
    bf = ml_dtypes.bfloat16
    f8 = ml_dtypes.float8_e4m3
    w_qkv = np.asarray(w_qkv, np.float32)
    w_dw = np.asarray(w_dw, np.float32).reshape(QKV, 9)  # tap idx = 3*(dy+1)+(dx+1)
    w_proj = np.asarray(w_proj, np.float32)
    temp = np.asarray(temperature, np.float32).reshape(HEADS)

    # q,k GEMM weights in fp8: DoubleRow pair for input ctiles 0,1 plus a
    # plain block for ctile 2; v GEMM weights stay bf16.
    wqdr = np.zeros((128, 6 * 256), np.float32)
    wq2 = np.zeros((128, 6 * 128), np.float32)
    for ot in range(6):
        for e in range(2):
            # [ki, e, m] = W[ot*128+m, e*128+ki]
            wqdr[:, ot * 256 + e * 128:ot * 256 + e * 128 + 128] = \
                w_qkv[ot * 128:(ot + 1) * 128, e * 128:(e + 1) * 128].T
        wq2[:, ot * 128:(ot + 1) * 128] = \
            w_qkv[ot * 128:(ot + 1) * 128, 256:384].T
    wqdr = wqdr.astype(f8)
    wq2 = wq2.astype(f8)
    wqv = np.ascontiguousarray(w_qkv[768:1152].T).astype(bf)  # [c, m]

    wdiag = np.zeros((128, 81 * 128), np.float32)
    wdwps = np.zeros((128, 81), np.float32)
    for ct in range(9):
        for tap in range(9):
            wv = w_dw[ct * 128:(ct + 1) * 128, tap]
            blk = (ct * 9 + tap) * 128
            wdiag[np.arange(128), blk + np.arange(128)] = wv
            wdwps[:, ct * 9 + tap] = wv
    wdiag = wdiag.astype(bf)

    wprojT = np.ascontiguousarray(w_proj.T).astype(bf)  # [c, o]

    tmprow = np.repeat(temp, CH).reshape(1, DIM).astype(np.float32)
    return {
        "wqdr": wqdr, "wq2": wq2, "wqv": wqv,
        "wdiag": wdiag, "wdwps": wdwps.astype(np.float32),
        "wprojT": wprojT, "tmprow": tmprow,
    }


def _make_in_maps(x, w_qkv, w_dw, w_proj, temperature):
    bf = ml_dtypes.bfloat16
    x = np.asarray(x, np.float32)
    shared = _prepare_shared_weights(w_qkv, w_dw, w_proj, temperature)
    in_maps = []
    for core in range(N_CORES):
        b, half = core // 2, core % 2
        h0 = half * 64
        xp = np.zeros((DIM, ROWS_LOC, W), np.float32)
        lo, hi = h0 - 1, h0 + 65
        slo, shi = max(lo, 0), min(hi, H)
        xp[:, slo - lo:shi - lo, :] = x[b, :, slo:shi, :]
        xf = xp.reshape(DIM, NLOC)
        in_maps.append({
            "xl": xf.astype(bf),
            "xl8": xf.astype(ml_dtypes.float8_e4m3),
            **shared,
        })
    return in_maps


def _get_nc():
    if "nc" not in _CACHE:
        nc = _build_nc()
        split_multiwaits(nc)
        _CACHE["nc"] = nc
    return _CACHE["nc"]


def _assemble(results):
    out = np.empty((B, DIM, H, W), np.float32)
    for core in range(N_CORES):
        b, half = core // 2, core % 2
        y = results[core]["yout"].astype(np.float32).reshape(DIM, 64, W)
        out[b, :, half * 64:half * 64 + 64, :] = y
    return out


def kernel(x, w_qkv, w_dw, w_proj, temperature):
    from concourse.bass_utils import run_bass_kernel_spmd

    in_maps = _make_in_maps(x, w_qkv, w_dw, w_proj, temperature)
    nc = _get_nc()
    res = run_bass_kernel_spmd(nc, in_maps, core_ids=list(range(N_CORES)))
    _CACHE["last_results"] = res
    return _assemble(res.results)


def benchmark(x, w_qkv, w_dw, w_proj, temperature, iters=6):
    """Compile once, execute `iters` times with device-resident inputs.
    Returns (output ndarray, per-iteration wall times in ns)."""
    import time
    import jax
    import jax.numpy as jnp
    from jax.sharding import Mesh, PartitionSpec, NamedSharding
    from jax.experimental.shard_map import shard_map
    from concourse import bass2jax
    from concourse.bass2jax import _bass_exec_p, install_neuronx_cc_hook
    import concourse.mybir as mb

    install_neuronx_cc_hook()
    in_maps = _make_in_maps(x, w_qkv, w_dw, w_proj, temperature)
    nc = _get_nc()

    in_names, out_names, out_avals = [], [], []
    for alloc in nc.m.functions[0].allocations:
        if not isinstance(alloc, mb.MemoryLocationSet):
            continue
        name = alloc.memorylocations[0].name
        if alloc.kind == "ExternalInput":
            if nc.partition_id_tensor is None or name != nc.partition_id_tensor.name:
                in_names.append(name)
        elif alloc.kind == "ExternalOutput":
            out_names.append(name)
            out_avals.append(
                jax.core.ShapedArray(tuple(alloc.tensor_shape), mb.dt.np(alloc.dtype))
            )
    n_params = len(in_names)
    zero_outs = [np.zeros(a.shape, a.dtype) for a in out_avals]
    all_in_names = list(in_names) + list(out_names)
    if nc.partition_id_tensor is not None:
        all_in_names.append(nc.partition_id_tensor.name)

    donate = tuple(range(n_params, n_params + len(out_names)))

    def _body(*args):
        operands = list(args)
        if nc.partition_id_tensor is not None:
            operands.append(bass2jax.partition_id_tensor())
        return tuple(
            _bass_exec_p.bind(
                *operands,
                out_avals=tuple(out_avals),
                in_names=tuple(all_in_names),
                out_names=tuple(out_names),
                lowering_input_output_aliases=(),
                sim_require_finite=True,
                sim_require_nnan=True,
                nc=nc,
            )
        )

    devices = jax.devices()[:N_CORES]
    mesh = Mesh(np.asarray(devices), ("core",))
    in_specs = (PartitionSpec("core"),) * (n_params + len(out_names))
    out_specs = (PartitionSpec("core"),) * len(out_names)
    fn = jax.jit(
        shard_map(_body, mesh=mesh, in_specs=in_specs, out_specs=out_specs,
                  check_rep=False),
        donate_argnums=donate, keep_unused=True,
    )

    sh = NamedSharding(mesh, PartitionSpec("core"))
    concat_in = [
        jax.device_put(
            np.concatenate([np.asarray(in_maps[c][n]) for c in range(N_CORES)], 0), sh
        )
        for n in in_names
    ]
    zsets = [
        [jax.device_put(np.zeros((N_CORES * z.shape[0], *z.shape[1:]), z.dtype), sh)
         for z in zero_outs]
        for _ in range(iters)
    ]

    times = []
    out_arrs = None
    for it in range(iters):
        for a in concat_in:
            a.block_until_ready()
        t0 = time.perf_counter_ns()
        res = fn(*concat_in, *zsets[it])
        for r in res:
            r.block_until_ready()
        times.append(time.perf_counter_ns() - t0)
        if it == iters - 1:
            out_arrs = res
    results = [
        {n: np.asarray(out_arrs[i]).reshape(N_CORES, *out_avals[i].shape)[c]
         for i, n in enumerate(out_names)}
        for c in range(N_CORES)
    ]
    return _assemble(results), times
